# revision 1
# baseline (speedup 1.0000x reference)
"""Multi-head cross-attention (B=2, Tq=Tk=2048, D=1024, H=16) on 8 TRN2 cores.

Sharding: core c handles batch b=c//4 and query rows 512*(c%4) .. +512 of that
batch (data-parallel over batch x query blocks).  Each core computes its
batch's K/V projections locally (duplicated across the 4 cores of a batch
group), runs attention for its 512 query rows over all 16 heads, then the
output projection + bias + residual + LayerNorm for its rows.  No collectives.

On-chip layout notes:
  - Activations transposed on the PE (bf16) so the d_model contraction sits on
    SBUF partitions for every projection matmul.
  - Attention uses the "scoresT" layout: scoresT[k, q] = kT_h.T @ qT_h with
    K=64 contraction (two heads row-packed via tile_position), so softmax's
    exp can be evacuated by the scalar engine with the pad-mask folded into
    the per-partition (per-key) scale/bias of the activation op, and the AV
    matmul consumes exp output directly (lhsT = V, rhs = attnT).
  - Softmax denominators come from a ones-vector matmul col-packed next to AV
    (partition 64 of the AV psum tile); no max-subtraction (scores are small).
  - All matmuls in bf16 (fp32 PSUM accumulation).
"""

import numpy as np
import ml_dtypes

import concourse.bass as bass
import concourse.tile as tile
from concourse import mybir
from concourse.bass_utils import run_bass_kernel_spmd
from concourse.vector_clock import ScopedClock

B, TQ, TK, D, H, DH = 2, 2048, 2048, 1024, 16, 64
NC = 8
ROWS = (B * TQ) // NC  # 512 query rows per core
F32 = mybir.dt.float32
BF16 = mybir.dt.bfloat16
AF = mybir.ActivationFunctionType
ALU = mybir.AluOpType


def _install_drain_split_patch():
    """This container's walrus caps sync-waits at 1 per (non-EVSEM)
    instruction, but TileContext's tail drain attaches one wait per proc lane.
    Split the waits across a chain of Drain instructions on SP."""
    if getattr(tile.TileContext, "_drain_split_patched", False):
        return

    def _patched(self, tick_clock, wait_clock):
        drain_inst = self.nc.sync.drain()
        wait_clock.add_sem_waits(
            drain_inst.ins, ScopedClock({None: tick_clock.global_clock})
        )
        si = drain_inst.ins.sync_info
        waits = list(si.on_wait) if si is not None and si.on_wait else []
        if len(waits) > 1:
            si.on_wait = waits[:1]
            import bass_rust

            for i in range(1, len(waits)):
                d2 = self.nc.sync.drain()
                si2 = d2.ins.sync_info
                if si2 is None:
                    d2.ins.sync_info = bass_rust.SyncInfo(
                        on_wait=waits[i : i + 1], on_update=[]
                    )
                else:
                    si2.on_wait = waits[i : i + 1]
        self.nc.all_engine_barrier()
        assert self.sems is not None
        popped = self.nc._tile_sem_poison_stack.pop()
        assert popped is self._sem_poison
        self.nc.clear_and_free_semaphores(list(self.sems.allocated().values()))
        self.nc.all_engine_barrier()

    tile.TileContext._drain_and_barrier = _patched
    tile.TileContext._drain_split_patched = True


def _split_excess_waits(nc, max_waits=1):
    """This container's walrus caps sync-waits per instruction; Tile attaches
    several. Move excess waits onto EventSemaphore instructions inserted just
    before the overloaded instruction on the same engine (same AND semantics,
    sequential)."""
    import bass_rust

    ctr = 0
    for f in nc.m.functions:
        for blk in f.blocks:
            out = []
            changed = False
            for inst in blk.instructions:
                si = inst.sync_info
                waits = list(si.on_wait) if si is not None and si.on_wait else []
                if len(waits) > max_waits:
                    for w in waits[:-max_waits]:
                        ev = mybir.InstEventSemaphore(
                            name=f"evwsplit_{ctr}",
                            engine=inst.engine,
                            ins=[],
                            outs=[],
                            sync_info=bass_rust.SyncInfo(on_wait=[w], on_update=[]),
                        )
                        ctr += 1
                        out.append(ev)
                    si.on_wait = waits[-max_waits:]
                    changed = True
                out.append(inst)
            if changed:
                blk.instructions = out


def build_bass(reps=1, upto="FULL"):
    nc = bass.Bass(trn_type="TRN2")

    x_rows = nc.dram_tensor("x_rows", [ROWS, D], F32, kind="ExternalInput")
    ctx_in = nc.dram_tensor("ctx_in", [TK, D], F32, kind="ExternalInput")
    pm_in = nc.dram_tensor("pm_in", [TK], F32, kind="ExternalInput")
    wq_in = nc.dram_tensor("wq_in", [D, D], F32, kind="ExternalInput")
    wk_in = nc.dram_tensor("wk_in", [D, D], F32, kind="ExternalInput")
    wv_in = nc.dram_tensor("wv_in", [D, D], F32, kind="ExternalInput")
    wo_in = nc.dram_tensor("wo_in", [D, D], F32, kind="ExternalInput")
    bo_in = nc.dram_tensor("bo_in", [D], F32, kind="ExternalInput")
    ga_in = nc.dram_tensor("ga_in", [D], F32, kind="ExternalInput")
    be_in = nc.dram_tensor("be_in", [D], F32, kind="ExternalInput")
    id_in = nc.dram_tensor("id_in", [128, 128], BF16, kind="ExternalInput")
    out_rows = nc.dram_tensor("out_rows", [ROWS, D], F32, kind="ExternalOutput")

    KD = D // 128  # 8 k-tiles over d_model
    RT = ROWS // 128  # 4 query row tiles
    KT = TK // 128  # 16 key tiles

    import contextlib

    with tile.TileContext(nc) as tc:
        est = contextlib.ExitStack()
        with est:
            # ---- constants (live across reps) ----
            singles = est.enter_context(tc.tile_pool(name="singles", bufs=1))
            ident = singles.tile([128, 128], BF16)
            nc.sync.dma_start(ident[:], id_in[:])
            ones_sb = singles.tile([128, 1], BF16)
            nc.vector.memset(ones_sb[:], 1.0)
            eps_t = singles.tile([128, 1], F32)
            nc.vector.memset(eps_t[:], 1e-5)

            # pad mask -> per-key exp scale/bias: exp(s*0.125*m + (m-1)*1e10)
            pm_sb = singles.tile([128, KT], F32)
            nc.sync.dma_start(pm_sb[:], pm_in[:].rearrange("(t p) -> p t", p=128))
            pm_scale = singles.tile([128, KT], F32)
            nc.vector.tensor_scalar_mul(pm_scale[:], pm_sb[:], 0.125)
            pm_bias = singles.tile([128, KT], F32)
            nc.vector.tensor_scalar(
                pm_bias[:], pm_sb[:], 1.0e10, -1.0e10, op0=ALU.mult, op1=ALU.add
            )

            def bcast_load(dram_ap, nm):
                t = singles.tile([128, D], F32, tag=nm, name=nm)
                src = bass.AP(
                    tensor=dram_ap.tensor,
                    offset=dram_ap.offset,
                    ap=[[0, 128], *dram_ap.ap],
                )
                nc.sync.dma_start(t[:], src)
                return t

            bo_bc = bcast_load(bo_in[:], "bo_bc")
            ga_bc = bcast_load(ga_in[:], "ga_bc")
            be_bc = bcast_load(be_in[:], "be_bc")

            for _rep in range(reps):
                _emit_rep(
                    nc, tc, contextlib, KD, RT, KT,
                    x_rows, ctx_in, wq_in, wk_in, wv_in, wo_in, out_rows,
                    ident, ones_sb, eps_t, pm_sb,
                    bo_bc, ga_bc, be_bc, upto,
                )

    _split_excess_waits(nc)
    return nc


def _emit_rep(
    nc, tc, contextlib, KD, RT, KT,
    x_rows, ctx_in, wq_in, wk_in, wv_in, wo_in, out_rows,
    ident, ones_sb, eps_t, pm_sb, bo_bc, ga_bc, be_bc, upto="FULL",
):
    rst = contextlib.ExitStack()
    with rst:
        natf = rst.enter_context(tc.tile_pool(name="natf", bufs=4))
        small = rst.enter_context(tc.tile_pool(name="small", bufs=2))

        # ---- phase T: transpose X (rows chunk) and C (full batch ctx) ----
        ct_stack = contextlib.ExitStack()
        ct_pool = ct_stack.enter_context(tc.tile_pool(name="ct", bufs=KD, side="right"))
        xt_stack = contextlib.ExitStack()
        xt_pool = xt_stack.enter_context(tc.tile_pool(name="xt", bufs=KD, side="right"))
        nath_stack = contextlib.ExitStack()
        nath = nath_stack.enter_context(tc.tile_pool(name="nath", bufs=6, side="right"))
        XT = [xt_pool.tile([128, ROWS], BF16, tag="xt", name=f"XT{i}") for i in range(KD)]
        CT = [ct_pool.tile([128, TK], BF16, tag="ct", name=f"CT{i}") for i in range(KD)]

        with tc.tile_pool(name="pt", bufs=3, space="PSUM") as pt_pool:

            def transpose_block(src_dram, n_rows_tiles, dest, dest_off):
                for g in range(n_rows_tiles // 4):
                    nats = []
                    for r in range(4):
                        nf = natf.tile([128, D], F32, tag="natf")
                        nc.sync.dma_start(
                            nf[:], src_dram[(g * 4 + r) * 128 : (g * 4 + r + 1) * 128, :]
                        )
                        nh = nath.tile([128, D], BF16, tag="nath")
                        nc.scalar.copy(nh[:], nf[:])
                        nats.append(nh)
                    for dt in range(KD):
                        ptile = pt_pool.tile([128, 512], BF16, tag="pt")
                        for r in range(4):
                            nc.tensor.transpose(
                                ptile[:, r * 128 : (r + 1) * 128],
                                nats[r][:, dt * 128 : (dt + 1) * 128],
                                ident[:],
                            )
                        nc.vector.tensor_copy(
                            dest[dt][:, dest_off + g * 512 : dest_off + (g + 1) * 512],
                            ptile[:],
                        )

            transpose_block(x_rows, RT, XT, 0)
            transpose_block(ctx_in, KT, CT, 0)
        nath_stack.close()
        if upto == "T":
            xt_stack.close()
            ct_stack.close()
            return

        wstage = rst.enter_context(tc.tile_pool(name="wstage", bufs=2))
        wh_pool = rst.enter_context(tc.tile_pool(name="wh", bufs=10))

        def load_weights_bf16(w_dram, tag):
            tiles = []
            for k in range(KD):
                wf = wstage.tile([128, D], F32, tag="wstage")
                nc.sync.dma_start(wf[:], w_dram[k * 128 : (k + 1) * 128, :])
                wt = wh_pool.tile([128, D], BF16, tag="wh")
                nc.vector.tensor_copy(wt[:], wf[:])
                tiles.append(wt)
            return tiles

        pp_stack = contextlib.ExitStack()
        with pp_stack:
            pp_pool = pp_stack.enter_context(
                tc.tile_pool(name="pp", bufs=2, space="PSUM")
            )

            # ---- phase Q: qT[dq, rows] = wq.T @ X.T ----
            wqh = load_weights_bf16(wq_in, "wq")
            qt_pool = rst.enter_context(tc.tile_pool(name="qt", bufs=KD))
            qT = [qt_pool.tile([128, ROWS], BF16, tag="qt", name=f"qT{i}") for i in range(KD)]
            for m in range(KD):
                ps = pp_pool.tile([128, 512], F32, tag="pp")
                for k in range(KD):
                    nc.tensor.matmul(
                        ps[:],
                        wqh[k][:, m * 128 : (m + 1) * 128],
                        XT[k][:],
                        start=(k == 0),
                        stop=(k == KD - 1),
                    )
                nc.vector.tensor_copy(qT[m][:], ps[:])
            xt_stack.close()

            # ---- phase K: kT[dk, keys] = wk.T @ C.T ----
            wkh = load_weights_bf16(wk_in, "wk")
            kt_pool = rst.enter_context(tc.tile_pool(name="kt", bufs=KD))
            kT = [kt_pool.tile([128, TK], BF16, tag="kt", name=f"kTt{i}") for i in range(KD)]
            for m in range(KD):
                for ncol in range(TK // 512):
                    ps = pp_pool.tile([128, 512], F32, tag="pp")
                    for k in range(KD):
                        nc.tensor.matmul(
                            ps[:],
                            wkh[k][:, m * 128 : (m + 1) * 128],
                            CT[k][:, ncol * 512 : (ncol + 1) * 512],
                            start=(k == 0),
                            stop=(k == KD - 1),
                        )
                    nc.vector.tensor_copy(
                        kT[m][:, ncol * 512 : (ncol + 1) * 512], ps[:]
                    )

            # ---- phase V: V_aug[keys, h, 65] = pad_mask * (C @ wv | ones) ----
            # col 64 of each head's 65-wide group holds the (masked) ones used
            # for the softmax denominator; V rows are pre-scaled by the mask so
            # exp needs no per-key bias (binary pad-mask semantics).
            wvh = load_weights_bf16(wv_in, "wv")
            v_pool = rst.enter_context(tc.tile_pool(name="v", bufs=KT))
            V = [
                v_pool.tile([128, H, DH + 1], BF16, tag="v", name=f"Vt{i}")
                for i in range(KT)
            ]
            for mk in range(KT):
                for ncol in range(D // 512):
                    ps = pp_pool.tile([128, 512], F32, tag="pp")
                    for k in range(KD):
                        nc.tensor.matmul(
                            ps[:],
                            CT[k][:, mk * 128 : (mk + 1) * 128],
                            wvh[k][:, ncol * 512 : (ncol + 1) * 512],
                            start=(k == 0),
                            stop=(k == KD - 1),
                        )
                    nc.vector.tensor_scalar_mul(
                        V[mk][:, ncol * 8 : (ncol + 1) * 8, 0:DH],
                        ps[:].rearrange("p (h d) -> p h d", d=DH),
                        pm_sb[:, mk : mk + 1],
                    )
                nc.vector.memset(V[mk][:, :, DH : DH + 1], 0.0)
                nc.vector.tensor_scalar(
                    V[mk][:, :, DH : DH + 1],
                    V[mk][:, :, DH : DH + 1],
                    1.0,
                    pm_sb[:, mk : mk + 1],
                    op0=ALU.mult,
                    op1=ALU.add,
                )

            ct_stack.close()  # context transpose no longer needed
            if upto == "QKV":
                return

            # ---- attention ----
            woh = load_weights_bf16(wo_in, "wo")
            avt_pool = rst.enter_context(tc.tile_pool(name="avt", bufs=KD))
            attn_pool = rst.enter_context(tc.tile_pool(name="attn", bufs=6))
            dram_pool = rst.enter_context(
                tc.tile_pool(name="dscratch", bufs=2, space="DRAM")
            )
            avT = [avt_pool.tile([128, ROWS], BF16, tag="avt", name=f"avT{i}") for i in range(KD)]
            with (
                tc.tile_pool(name="sc", bufs=2, space="PSUM") as sc_pool,
                tc.tile_pool(name="pav", bufs=2, space="PSUM") as pav_pool,
            ):
                # heads processed in pairs: even head on PE rows 0-63, odd
                # head on rows 64-127, adjacent in issue order so the array's
                # row-group packing runs both score matmuls concurrently.
                for m in range(H // 2):
                    avs = [
                        pav_pool.tile([128, 512], F32, tag="pav", name=f"av{m}_{i}")
                        for i in range(2)
                    ]
                    for kt2 in range(KT // 2):
                        sps = [
                            sc_pool.tile([128, 1024], F32, tag="sc", name=f"sp{m}_{kt2}_{i}")
                            for i in range(2)
                        ]
                        for half in range(2):
                            kt = 2 * kt2 + half
                            for i in range(2):
                                off = 64 * i
                                nc.tensor.matmul(
                                    sps[i][:, half * 512 : (half + 1) * 512],
                                    kT[m][off : off + 64, kt * 128 : (kt + 1) * 128],
                                    qT[m][off : off + 64, :],
                                    start=True,
                                    stop=True,
                                    tile_position=(off, 0),
                                )
                        ats = []
                        for i in range(2):
                            at = attn_pool.tile(
                                [128, 1024], BF16, tag="attn", name=f"at{m}_{kt2}_{i}"
                            )
                            nc.scalar.activation(at[:], sps[i][:], AF.Exp, scale=0.125)
                            ats.append(at)
                        for half in range(2):
                            kt = 2 * kt2 + half
                            for i in range(2):
                                h = 2 * m + i
                                nc.tensor.matmul(
                                    avs[i][0:65, :],
                                    V[kt][:, h, 0 : DH + 1],
                                    ats[i][:, half * 512 : (half + 1) * 512],
                                    start=(kt == 0),
                                    stop=(kt == KT - 1),
                                    tile_position=(0, 0),
                                    skip_group_check=True,
                                )
                    for i in range(2):
                        h = 2 * m + i
                        off = 64 * i
                        av = avs[i]
                        recip = small.tile([1, 512], F32, tag="recip")
                        nc.vector.reciprocal(recip[:], av[64:65, :])
                        dstage = dram_pool.tile([1, 512], F32, tag="dstage")
                        nc.sync.dma_start(dstage[:], recip[:])
                        denb = small.tile([64, 512], F32, tag="denb")
                        dsrc = bass.AP(
                            tensor=dstage[:].tensor,
                            offset=dstage[:].offset,
                            ap=[[0, 64], dstage[:].ap[-1]],
                        )
                        nc.sync.dma_start(denb[:], dsrc)
                        nc.vector.tensor_tensor(
                            avT[m][off : off + 64, :],
                            av[0:64, :],
                            denb[:],
                            op=ALU.mult,
                        )

            if upto == "ATTN":
                return

            # ---- output projection + bias + residual + layernorm ----
            y_pool = rst.enter_context(tc.tile_pool(name="y", bufs=2))
            for rt in range(RT):
                rx = natf.tile([128, D], F32, tag="natf")
                nc.sync.dma_start(rx[:], x_rows[rt * 128 : (rt + 1) * 128, :])
                y = y_pool.tile([128, D], F32, tag="y")
                for ncol in range(D // 512):
                    pj = pp_pool.tile([128, 512], F32, tag="pp")
                    for k in range(KD):
                        nc.tensor.matmul(
                            pj[:],
                            avT[k][:, rt * 128 : (rt + 1) * 128],
                            woh[k][:, ncol * 512 : (ncol + 1) * 512],
                            start=(k == 0),
                            stop=(k == KD - 1),
                        )
                    nc.vector.tensor_tensor(
                        y[:, ncol * 512 : (ncol + 1) * 512],
                        pj[:],
                        rx[:, ncol * 512 : (ncol + 1) * 512],
                        op=ALU.add,
                    )
                nc.vector.tensor_tensor(y[:], y[:], bo_bc[:], op=ALU.add)
                stats = small.tile([128, 2, 6], F32, tag="stats")
                nc.vector.bn_stats(stats[:, 0, :], y[:, 0:512])
                nc.vector.bn_stats(stats[:, 1, :], y[:, 512:1024])
                mv = small.tile([128, 2], F32, tag="mv")
                nc.vector.bn_aggr(mv[:], stats[:])
                sq = small.tile([128, 1], F32, tag="sq")
                nc.scalar.activation(
                    sq[:], mv[:, 1:2], AF.Sqrt, bias=eps_t[:], scale=1.0
                )
                rstd = small.tile([128, 1], F32, tag="rstd")
                nc.vector.reciprocal(rstd[:], sq[:])
                nc.vector.tensor_scalar(
                    y[:],
                    y[:],
                    mv[:, 0:1],
                    rstd[:],
                    op0=ALU.subtract,
                    op1=ALU.mult,
                )
                nc.vector.tensor_tensor(y[:], y[:], ga_bc[:], op=ALU.mult)
                nc.vector.tensor_tensor(y[:], y[:], be_bc[:], op=ALU.add)
                nc.sync.dma_start(out_rows[rt * 128 : (rt + 1) * 128, :], y[:])


_BUILT = None


def _get_built():
    global _BUILT
    if _BUILT is None:
        _install_drain_split_patch()
        _BUILT = build_bass()
    return _BUILT


def make_in_maps(target, context, pad_mask, wq, wk, wv, wo, bo, ln_gamma, ln_beta):
    ident = np.eye(128, dtype=ml_dtypes.bfloat16)
    shared = {
        "wq_in": np.ascontiguousarray(wq, dtype=np.float32),
        "wk_in": np.ascontiguousarray(wk, dtype=np.float32),
        "wv_in": np.ascontiguousarray(wv, dtype=np.float32),
        "wo_in": np.ascontiguousarray(wo, dtype=np.float32),
        "bo_in": np.ascontiguousarray(bo, dtype=np.float32),
        "ga_in": np.ascontiguousarray(ln_gamma, dtype=np.float32),
        "be_in": np.ascontiguousarray(ln_beta, dtype=np.float32),
        "id_in": ident,
    }
    in_maps = []
    for c in range(NC):
        b = c // (NC // B)
        j = c % (NC // B)
        m = dict(shared)
        m["x_rows"] = np.ascontiguousarray(
            target[b, j * ROWS : (j + 1) * ROWS, :], dtype=np.float32
        )
        m["ctx_in"] = np.ascontiguousarray(context[b], dtype=np.float32)
        m["pm_in"] = np.ascontiguousarray(pad_mask[b], dtype=np.float32)
        in_maps.append(m)
    return in_maps


def kernel(target, context, pad_mask, wq, wk, wv, wo, bo, ln_gamma, ln_beta):
    nc = _get_built()
    in_maps = make_in_maps(
        target, context, pad_mask, wq, wk, wv, wo, bo, ln_gamma, ln_beta
    )
    res = run_bass_kernel_spmd(nc, in_maps, core_ids=list(range(NC)), trace=False)
    out = np.empty((B, TQ, D), dtype=np.float32)
    for c in range(NC):
        b = c // (NC // B)
        j = c % (NC // B)
        out[b, j * ROWS : (j + 1) * ROWS, :] = res.results[c]["out_rows"]
    return out

